# revision 15
# baseline (speedup 1.0000x reference)
"""Trainium2 Bass kernel for nn_MultiHeadAttention (B=4, S=2048, D=768, H=12).

Sharding: 8 cores = 4 batches x 2 head-groups (6 heads each).
Per core, everything is computed in transposed layout:
  QT = Wq_g @ x_b^T            [384, 2048]   (bf16, bias folded)
  KT = Wk_g @ x_b^T            [384, 2048]   (bf16, bias folded)
  V  = x_b @ Wv_g^T            [2048, 6*128] (bf16; per-head 64 data + 64 ones cols)
  per head pair hp, q-chunk qc:
    S^T[k,q]  = K_h Q_h^T      (PE, k on partitions; 2 heads row-tiled concurrent)
    E = exp(S^T/64)            (ScalarE, no max-subtraction: |S/64| < ~1)
    attT_aug  = [V_h | 1]^T E  [128, 512]; rows 64-127 = softmax denominator
    attn = attT * approx_recip(denom)   (custom DVE recip + DVE mult)
  outT_partial = Wo_g @ attn   [768, 2048] bf16 (+bo on g==0 cores)
Host sums the two partial outT per batch (f32) and transposes back.

v2: emission order interleaves projection/V units between attention units so
ScalarE (the exp bottleneck, ~220us/core) saturates from ~10us and the PE
stays dense (HAM warm); bit-exact reciprocal -> reciprocal_approx_fast;
outT written bf16.

Self-contained: hardcodes all shapes; only imports concourse + numpy.
"""

import os
import sys

import numpy as np
import ml_dtypes

if "/opt/trn_rl_repo" not in sys.path:
    sys.path.insert(0, "/opt/trn_rl_repo")

import concourse.bass as bass
import concourse.bacc as bacc
import concourse.mybir as mybir
import concourse.tile as tile
from concourse.bass_utils import run_bass_kernel_spmd

# Problem dims
B, S, DM, NH, DK = 4, 2048, 768, 12, 64
NCORES = 8
HLOC = 6          # heads per core
GD = HLOC * DK    # 384 head dims per core
P = 128
NXT = DM // P     # 6 contraction tiles over d_model
NPT = GD // P     # 3 partition tiles over per-core head dims
NKT = S // P      # 16 k tiles
QC = 512          # q chunk
NQC = S // QC     # 4
VROW = HLOC * 2 * DK  # 768: per head, 64 data cols + 64 ones cols (denom replication)

F32 = mybir.dt.float32
BF16 = mybir.dt.bfloat16
EXP = mybir.ActivationFunctionType.Exp
NPBF16 = ml_dtypes.bfloat16

_NC_CACHE = {}


def build_nc():
    nc = bacc.Bacc()

    xT = nc.declare_dram_parameter("xT", [DM, S], BF16, isOutput=False)
    wqT = nc.declare_dram_parameter("wqT", [DM, GD], BF16, isOutput=False)
    wkT = nc.declare_dram_parameter("wkT", [DM, GD], BF16, isOutput=False)
    wvT = nc.declare_dram_parameter("wvT", [DM, GD], BF16, isOutput=False)
    woT = nc.declare_dram_parameter("woT", [GD, DM], BF16, isOutput=False)
    pb = nc.declare_dram_parameter("pb", [P, 12], F32, isOutput=False)
    rcb = nc.declare_dram_parameter("rcb", [1, 512], BF16, isOutput=False)
    outT = nc.declare_dram_parameter("outT", [DM, S], BF16, isOutput=True)

    with tile.TileContext(nc) as tc:
        with (
            nc.allow_low_precision(reason="bf16 matmul pipeline is intended"),
            tc.tile_pool(name="persist", bufs=1) as pp,
            tc.tile_pool(name="psum", bufs=1, space=bass.MemorySpace.PSUM) as psp,
            tc.tile_pool(name="work", bufs=1) as wp,
        ):
            # ---- loads; ordered so K0-chunk-qc0 (the first PE work and the
            # feeder of the first exp) has its deps after ~12 small DMAs ----
            xt, wq_t, wk_t, wv_t = [], [], [], []
            for i in range(NXT):
                t = pp.tile([P, GD], BF16, tag=f"wk{i}", name=f"wk{i}")
                nc.sync.dma_start(t[:], wkT[i * P : (i + 1) * P, :])
                wk_t.append(t)
                tx = pp.tile([P, S], BF16, tag=f"xt{i}", name=f"xt{i}")
                nc.sync.dma_start(tx[:, 0:QC], xT[i * P : (i + 1) * P, 0:QC])
                xt.append(tx)
            pb_t = pp.tile([P, 12], F32, tag="pb", name="pb_t")
            nc.sync.dma_start(pb_t[:], pb[:])
            for i in range(NXT):
                t = pp.tile([P, GD], BF16, tag=f"wq{i}", name=f"wq{i}")
                nc.sync.dma_start(t[:], wqT[i * P : (i + 1) * P, :])
                wq_t.append(t)
            # remaining x^T columns in qc-sized chunks, qc1 first (earliest
            # consumers: qproj(0,1) and the st>=4 V tiles)
            for qc in range(1, NQC):
                for i in range(NXT):
                    nc.sync.dma_start(
                        xt[i][:, qc * QC : (qc + 1) * QC],
                        xT[i * P : (i + 1) * P, qc * QC : (qc + 1) * QC],
                    )
                if qc == 1:
                    rcb_t = pp.tile([1, 512], BF16, tag="rcb", name="rcb_t")
                    nc.sync.dma_start(rcb_t[:], rcb[:])
                    for i in range(NXT):
                        t = pp.tile([P, GD], BF16, tag=f"wv{i}", name=f"wv{i}")
                        nc.sync.dma_start(t[:], wvT[i * P : (i + 1) * P, :])
                        wv_t.append(t)
            wo_t = []
            for j in range(NPT):
                t = pp.tile([P, DM], BF16, tag=f"wo{j}", name=f"wo{j}")
                nc.sync.dma_start(t[:], woT[j * P : (j + 1) * P, :])
                wo_t.append(t)

            bv_row = rcb_t[0:1, 0:GD]         # [1, 384]
            ones_row = rcb_t[0:1, GD:GD + P]  # [1, 128] of 1.0

            # ---- persistent result tiles ----
            QT, KT = [], []
            for nm, dst in (("QT", QT), ("KT", KT)):
                for pt in range(NPT):
                    t = pp.tile([P, S], BF16, tag=f"{nm}{pt}", name=f"{nm}{pt}")
                    dst.append(t)
            V = []
            for st in range(NKT):
                t = pp.tile([P, VROW], BF16, tag=f"V{st}", name=f"V{st}")
                V.append(t)
            attn = []
            for hp in range(NPT):
                t = pp.tile([P, S], BF16, tag=f"attn{hp}", name=f"attn{hp}")
                attn.append(t)

            # ---- emission helpers (Tile priority follows emission order) ----
            def proj1(nm, w, bcol, dst, pt, qc):
                """One [128, 512] chunk of Q^T or K^T projection.

                PSUM from the "ab" tag so the "st" slots stay dedicated to
                the S^T -> exp stream (ScalarE pacing).
                """
                ps = psp.tile([P, QC], F32, tag="ab", bufs=4,
                              name=f"ps_{nm}{pt}_{qc}")
                for kt in range(NXT):
                    nc.tensor.matmul(
                        ps[:],
                        w[kt][:, pt * P : (pt + 1) * P],
                        xt[kt][:, qc * QC : (qc + 1) * QC],
                        start=(kt == 0),
                        stop=(kt == NXT - 1),
                    )
                nc.vector.tensor_scalar_add(
                    dst[pt][:, qc * QC : (qc + 1) * QC],
                    ps[:],
                    pb_t[:, bcol + pt : bcol + pt + 1],
                )

            def kproj(pt, qc):
                proj1("KT", wk_t, 3, KT, pt, qc)

            def qproj(pt, qc):
                proj1("QT", wq_t, 0, QT, pt, qc)

            def vproj(st):
                """V tile for kv block st: [128 tokens, 6*(64 data + 64 ones)]."""
                ps = psp.tile([P, QC], F32, tag="ab", bufs=4, name=f"ps_v{st}")
                for kt in range(NXT):
                    nc.tensor.matmul(
                        ps[:, 0:GD],
                        xt[kt][:, st * P : (st + 1) * P],
                        wv_t[kt][:],
                        start=(kt == 0),
                        stop=False,
                    )
                nc.tensor.matmul(ps[:, 0:GD], ones_row, bv_row, start=False, stop=True)
                vv = V[st].rearrange("p (h c) -> p h c", h=HLOC)
                nc.vector.tensor_copy(
                    vv[:, :, 0:DK],
                    ps[:, 0:GD].rearrange("p (h c) -> p h c", h=HLOC),
                )
                nc.vector.memset(vv[:, :, DK : 2 * DK], 1.0)

            def oproj(oqc, mts=range(NXT)):
                oqsl = slice(oqc * QC, (oqc + 1) * QC)
                for mt in mts:
                    po = psp.tile([P, QC], F32, tag="ab", bufs=4, name=f"po{mt}_{oqc}")
                    for j in range(NPT):
                        nc.tensor.matmul(
                            po[:],
                            wo_t[j][:, mt * P : (mt + 1) * P],
                            attn[j][:, oqsl],
                            start=(j == 0),
                            stop=(j == NPT - 1),
                        )
                    osb = wp.tile([P, QC], BF16, tag="os", bufs=4, name=f"os{mt}_{oqc}")
                    nc.vector.tensor_scalar_add(osb[:], po[:], pb_t[:, 6 + mt : 7 + mt])
                    nc.sync.dma_start(outT[mt * P : (mt + 1) * P, oqsl], osb[:])

            def chain(qc, hp, inject=None):
                """S^T + exp chain for one (q-chunk, head-pair) unit.

                This is ScalarE's feeder; emitted as one contiguous
                high-priority block so nothing outranks the next S^T pair on
                the PE. Returns unit state for attv()/norm().

                inject: {ktp: [closure]} — emissions forced before that ktp's
                S^T (hard RAW ordering, e.g. the KT chunk it reads).
                """
                psA = psp.tile([P, QC], F32, tag="ab", bufs=4, name=f"att_a{hp}_{qc}")
                psB = psp.tile([P, QC], F32, tag="ab", bufs=4, name=f"att_b{hp}_{qc}")
                es = []
                qsl = slice(qc * QC, (qc + 1) * QC)
                for ktp in range(NKT // 2):
                    for f in (inject or {}).get(ktp, ()):
                        f()
                    stA = psp.tile([P, 2 * QC], F32, tag="st", bufs=2,
                                   name=f"stA{hp}_{qc}_{ktp}")
                    stB = psp.tile([P, 2 * QC], F32, tag="st", bufs=2,
                                   name=f"stB{hp}_{qc}_{ktp}")
                    for j in range(2):
                        kt = 2 * ktp + j
                        ksl = slice(kt * P, (kt + 1) * P)
                        jsl = slice(j * QC, (j + 1) * QC)
                        # S^T = K_h @ Q_h^T, two heads row-tiled in the PE
                        nc.tensor.matmul(
                            stA[:, jsl], KT[hp][0:DK, ksl], QT[hp][0:DK, qsl]
                        )
                        nc.tensor.matmul(
                            stB[:, jsl], KT[hp][DK:P, ksl], QT[hp][DK:P, qsl]
                        )
                    eA = wp.tile([P, 2 * QC], BF16, tag="E", bufs=32,
                                 name=f"eA{hp}_{qc}_{ktp}")
                    eB = wp.tile([P, 2 * QC], BF16, tag="E", bufs=32,
                                 name=f"eB{hp}_{qc}_{ktp}")
                    nc.scalar.activation(eA[:], stA[:], EXP, scale=1.0 / DK)
                    nc.scalar.activation(eB[:], stB[:], EXP, scale=1.0 / DK)
                    es.append((eA, eB))
                return {"qc": qc, "hp": hp, "psA": psA, "psB": psB, "es": es}

            def attv(u, lo, hi, v_pop=False):
                """attV accumulation for kv tiles [lo, hi) of unit u. Emitted
                after the NEXT unit's chain so the chain wins PE priority."""
                hA, hB = 2 * u["hp"], 2 * u["hp"] + 1
                for ktp in range(lo // 2, hi // 2):
                    eA, eB = u["es"][ktp]
                    if v_pop:
                        vproj(2 * ktp)
                        vproj(2 * ktp + 1)
                    for j in range(2):
                        kt = 2 * ktp + j
                        jsl = slice(j * QC, (j + 1) * QC)
                        nc.tensor.matmul(
                            u["psA"][:],
                            V[kt][:, hA * 2 * DK : (hA + 1) * 2 * DK],
                            eA[:, jsl],
                            start=(kt == 0),
                            stop=(kt == NKT - 1),
                            skip_group_check=True,
                        )
                        nc.tensor.matmul(
                            u["psB"][:],
                            V[kt][:, hB * 2 * DK : (hB + 1) * 2 * DK],
                            eB[:, jsl],
                            start=(kt == 0),
                            stop=(kt == NKT - 1),
                            skip_group_check=True,
                        )

            def norm(u):
                # attn rows = att * recip(denom); denom replicated in rows
                # 64-127 (ones cols of V). DVE only — no PE contention.
                qc, hp = u["qc"], u["hp"]
                qsl = slice(qc * QC, (qc + 1) * QC)
                nA = wp.tile([DK, QC], F32, tag="nm", bufs=4, name=f"nA{hp}_{qc}")
                nB = wp.tile([DK, QC], F32, tag="nm", bufs=4, name=f"nB{hp}_{qc}")
                nc.vector.reciprocal(nA[:], u["psA"][DK:P, :])
                nc.vector.reciprocal(nB[:], u["psB"][DK:P, :])
                nc.vector.tensor_mul(attn[hp][0:DK, qsl], u["psA"][0:DK, :], nA[:])
                nc.vector.tensor_mul(attn[hp][DK:P, qsl], u["psB"][0:DK, :], nB[:])

            # ---- emission schedule ----
            # hp-outer / qc-inner unit order. Chains are emitted ONE AHEAD of
            # their window's filler content: everything in window(w) is
            # emitted after chain(w+1), so the S^T -> exp stream always holds
            # top PE priority and filler work (attV halves, projections,
            # o-proj) soaks the PE idle underneath.
            units = [(qc, hp) for hp in range(NPT) for qc in range(NQC)]
            NU = len(units)
            us = []

            def window(w):
                # q-projection feeding chain(w+2), two windows ahead of use
                if w + 2 < NU:
                    qproj(units[w + 2][1], units[w + 2][0])
                # this unit's full attV + normalize: keeps the accumulator
                # pair's PSUM residency to ~one window so the "ab" slots stay
                # available for projection/o-proj transients
                attv(us[w], 0, NKT, v_pop=(w == 0))
                norm(us[w])
                # fillers: k-projections a few units ahead of their chain,
                # o-proj once a q-chunk's attn is complete (hp==2 stream)
                if w == 1:
                    kproj(1, 0), kproj(1, 1)
                elif w == 2:
                    kproj(1, 2), kproj(1, 3)
                elif w == 3:
                    kproj(2, 0), kproj(2, 1)
                elif w == 4:
                    kproj(2, 2), kproj(2, 3)
                elif w == 8:
                    oproj(0)
                elif w == 9:
                    oproj(1)
                elif w == 10:
                    oproj(2)
                elif w == 11:
                    oproj(3)

            kproj(0, 0)
            qproj(0, 0)
            for i, (qc, hp) in enumerate(units):
                inject = None
                if i == 0:
                    inject = {2: [lambda: kproj(0, 1)],
                              4: [lambda: kproj(0, 2)],
                              6: [lambda: kproj(0, 3)]}
                us.append(chain(qc, hp, inject))
                if i == 0:
                    qproj(units[1][1], units[1][0])
                else:
                    window(i - 1)
            window(NU - 1)

    nc.compile()
    return nc


def make_in_maps(x, Wq, bq, Wk, bk, Wv, bv, Wo, bo):
    in_maps = []
    for c in range(NCORES):
        b, g = c // 2, c % 2
        sl = slice(g * GD, (g + 1) * GD)
        pbv = np.zeros((P, 12), np.float32)
        for j in range(NPT):
            pbv[:, 0 + j] = bq[sl][j * P : (j + 1) * P]
            pbv[:, 3 + j] = bk[sl][j * P : (j + 1) * P]
        if g == 0:
            for j in range(NXT):
                pbv[:, 6 + j] = bo[j * P : (j + 1) * P]
        rcbv = np.zeros((1, 512), NPBF16)
        rcbv[0, :GD] = bv[sl].astype(NPBF16)
        rcbv[0, GD : GD + P] = NPBF16(1.0)
        in_maps.append(
            {
                "xT": np.ascontiguousarray(x[b].T).astype(NPBF16),
                "wqT": np.ascontiguousarray(Wq[sl, :].T).astype(NPBF16),
                "wkT": np.ascontiguousarray(Wk[sl, :].T).astype(NPBF16),
                "wvT": np.ascontiguousarray(Wv[sl, :].T).astype(NPBF16),
                "woT": np.ascontiguousarray(Wo[:, sl].T).astype(NPBF16),
                "pb": pbv,
                "rcb": rcbv,
            }
        )
    return in_maps


def kernel(x, Wq, bq, Wk, bk, Wv, bv, Wo, bo, _trace=False):
    x = np.asarray(x, np.float32)
    args = [np.asarray(a, np.float32) for a in (Wq, bq, Wk, bk, Wv, bv, Wo, bo)]
    if "nc" not in _NC_CACHE:
        _NC_CACHE["nc"] = build_nc()
    nc = _NC_CACHE["nc"]
    in_maps = make_in_maps(x, *args)
    res = run_bass_kernel_spmd(
        nc, in_maps, core_ids=list(range(NCORES)), trace=_trace
    )
    _NC_CACHE["last_result"] = res
    out = np.empty((B, S, DM), np.float32)
    for b in range(B):
        out[b] = (
            np.asarray(res.results[2 * b]["outT"], dtype=np.float32)
            + np.asarray(res.results[2 * b + 1]["outT"], dtype=np.float32)
        ).T
    return out


# revision 20
# speedup vs baseline: 1.1570x; 1.1570x over previous
"""Trainium2 Bass kernel for nn_MultiHeadAttention (B=4, S=2048, D=768, H=12).

Sharding: 8 cores = 4 batches x 2 head-groups (6 heads each).
Per core, everything is computed in transposed layout:
  QT = Wq_g @ x_b^T            [384, 2048]   (bf16, bias folded)
  KT = Wk_g @ x_b^T            [384, 2048]   (bf16, bias folded)
  V  = x_b @ Wv_g^T            [2048, 6*128] (bf16; per-head 64 data + 64 ones cols)
  per head pair hp, q-chunk qc:
    S^T[k,q]  = K_h Q_h^T      (PE, k on partitions; 2 heads row-tiled concurrent)
    E = exp(S^T/64)            (ScalarE, no max-subtraction: |S/64| < ~1)
    attT_aug  = [V_h | 1]^T E  [128, 512]; rows 64-127 = softmax denominator
    attn = attT * approx_recip(denom)   (custom DVE recip + DVE mult)
  outT_partial = Wo_g @ attn   [768, 2048] bf16 (+bo on g==0 cores)
Host sums the two partial outT per batch (f32) and transposes back.

v2: emission order interleaves projection/V units between attention units so
ScalarE (the exp bottleneck, ~220us/core) saturates from ~10us and the PE
stays dense (HAM warm); bit-exact reciprocal -> reciprocal_approx_fast;
outT written bf16.

Self-contained: hardcodes all shapes; only imports concourse + numpy.
"""

import os
import sys

import numpy as np
import ml_dtypes

if "/opt/trn_rl_repo" not in sys.path:
    sys.path.insert(0, "/opt/trn_rl_repo")

import concourse.bass as bass
import concourse.bacc as bacc
import concourse.mybir as mybir
import concourse.tile as tile
from concourse.bass_utils import run_bass_kernel_spmd

# Problem dims
B, S, DM, NH, DK = 4, 2048, 768, 12, 64
NCORES = 8
HLOC = 6          # heads per core
GD = HLOC * DK    # 384 head dims per core
P = 128
NXT = DM // P     # 6 contraction tiles over d_model
NPT = GD // P     # 3 partition tiles over per-core head dims
NKT = S // P      # 16 k tiles
QC = 512          # q chunk
NQC = S // QC     # 4
VROW = HLOC * 2 * DK  # 768: per head, 64 data cols + 64 ones cols (denom replication)

F32 = mybir.dt.float32
BF16 = mybir.dt.bfloat16
EXP = mybir.ActivationFunctionType.Exp
NPBF16 = ml_dtypes.bfloat16

_NC_CACHE = {}


def build_nc():
    nc = bacc.Bacc()

    xT = nc.declare_dram_parameter("xT", [DM, S], BF16, isOutput=False)
    wqT = nc.declare_dram_parameter("wqT", [DM, GD], BF16, isOutput=False)
    wkT = nc.declare_dram_parameter("wkT", [DM, GD], BF16, isOutput=False)
    wvT = nc.declare_dram_parameter("wvT", [DM, GD], BF16, isOutput=False)
    woT = nc.declare_dram_parameter("woT", [GD, DM], BF16, isOutput=False)
    pb = nc.declare_dram_parameter("pb", [P, 12], F32, isOutput=False)
    rcb = nc.declare_dram_parameter("rcb", [1, 512], BF16, isOutput=False)
    outT = nc.declare_dram_parameter("outT", [DM, S], BF16, isOutput=True)

    with tile.TileContext(nc) as tc:
        with (
            nc.allow_low_precision(reason="bf16 matmul pipeline is intended"),
            tc.tile_pool(name="persist", bufs=1) as pp,
            tc.tile_pool(name="psum", bufs=1, space=bass.MemorySpace.PSUM) as psp,
            tc.tile_pool(name="work", bufs=1) as wp,
        ):
            # ---- loads: one wide DMA per weight matrix / x^T q-chunk, so
            # the first projection's deps land after 2 DMAs instead of 12
            # (each dma_start costs ~650ns of Sync-queue issue time) ----
            def wide_load(name, dram, rows, cols):
                t = pp.tile([P, (rows // P) * cols], BF16, tag=name, name=name)
                nc.sync.dma_start(
                    t.rearrange("p (i c) -> p i c", c=cols),
                    dram.rearrange("(i p) c -> p i c", p=P),
                )
                return [t[:, i * cols : (i + 1) * cols] for i in range(rows // P)]

            xt_all = pp.tile([P, NXT * S], BF16, tag="xt", name="xt_all")
            xt = [xt_all[:, i * S : (i + 1) * S] for i in range(NXT)]
            xt3 = xt_all.rearrange("p (i s) -> p i s", s=S)
            xT3 = xT.rearrange("(i p) s -> p i s", p=P)

            wk_t = wide_load("wk", wkT, DM, GD)
            nc.sync.dma_start(xt3[:, :, 0:QC], xT3[:, :, 0:QC])
            pb_t = pp.tile([P, 12], F32, tag="pb", name="pb_t")
            nc.sync.dma_start(pb_t[:], pb[:])
            wq_t = wide_load("wq", wqT, DM, GD)
            nc.sync.dma_start(xt3[:, :, QC : 2 * QC], xT3[:, :, QC : 2 * QC])
            rcb_t = pp.tile([1, 512], BF16, tag="rcb", name="rcb_t")
            nc.sync.dma_start(rcb_t[:], rcb[:])
            wv_t = wide_load("wv", wvT, DM, GD)
            nc.sync.dma_start(xt3[:, :, 2 * QC : S], xT3[:, :, 2 * QC : S])
            wo_t = wide_load("wo", woT, GD, DM)

            bv_row = rcb_t[0:1, 0:GD]         # [1, 384]
            ones_row = rcb_t[0:1, GD:GD + P]  # [1, 128] of 1.0

            # ---- persistent result tiles ----
            QT, KT = [], []
            for nm, dst in (("QT", QT), ("KT", KT)):
                for pt in range(NPT):
                    t = pp.tile([P, S], BF16, tag=f"{nm}{pt}", name=f"{nm}{pt}")
                    dst.append(t)
            V = []
            for st in range(NKT):
                t = pp.tile([P, VROW], BF16, tag=f"V{st}", name=f"V{st}")
                V.append(t)
            attn = []
            for hp in range(NPT):
                t = pp.tile([P, S], BF16, tag=f"attn{hp}", name=f"attn{hp}")
                attn.append(t)

            # ---- emission helpers (Tile priority follows emission order) ----
            def proj1(nm, w, bcol, dst, pt, qc):
                """One [128, 512] chunk of Q^T or K^T projection.

                PSUM from the "ab" tag so the "st" slots stay dedicated to
                the S^T -> exp stream (ScalarE pacing).
                """
                ps = psp.tile([P, QC], F32, tag="ab", bufs=4,
                              name=f"ps_{nm}{pt}_{qc}")
                for kt in range(NXT):
                    nc.tensor.matmul(
                        ps[:],
                        w[kt][:, pt * P : (pt + 1) * P],
                        xt[kt][:, qc * QC : (qc + 1) * QC],
                        start=(kt == 0),
                        stop=(kt == NXT - 1),
                    )
                nc.vector.tensor_scalar_add(
                    dst[pt][:, qc * QC : (qc + 1) * QC],
                    ps[:],
                    pb_t[:, bcol + pt : bcol + pt + 1],
                )

            def kproj(pt, qc):
                proj1("KT", wk_t, 3, KT, pt, qc)

            def qproj(pt, qc):
                proj1("QT", wq_t, 0, QT, pt, qc)

            def vproj(st):
                """V tile for kv block st: [128 tokens, 6*(64 data + 64 ones)]."""
                ps = psp.tile([P, QC], F32, tag="ab", bufs=4, name=f"ps_v{st}")
                for kt in range(NXT):
                    nc.tensor.matmul(
                        ps[:, 0:GD],
                        xt[kt][:, st * P : (st + 1) * P],
                        wv_t[kt][:],
                        start=(kt == 0),
                        stop=False,
                    )
                nc.tensor.matmul(ps[:, 0:GD], ones_row, bv_row, start=False, stop=True)
                vv = V[st].rearrange("p (h c) -> p h c", h=HLOC)
                nc.vector.tensor_copy(
                    vv[:, :, 0:DK],
                    ps[:, 0:GD].rearrange("p (h c) -> p h c", h=HLOC),
                )
                nc.vector.memset(vv[:, :, DK : 2 * DK], 1.0)

            def oproj(oqc, mts=range(NXT)):
                oqsl = slice(oqc * QC, (oqc + 1) * QC)
                for mt in mts:
                    po = psp.tile([P, QC], F32, tag="ab", bufs=4, name=f"po{mt}_{oqc}")
                    for j in range(NPT):
                        nc.tensor.matmul(
                            po[:],
                            wo_t[j][:, mt * P : (mt + 1) * P],
                            attn[j][:, oqsl],
                            start=(j == 0),
                            stop=(j == NPT - 1),
                        )
                    osb = wp.tile([P, QC], BF16, tag="os", bufs=4, name=f"os{mt}_{oqc}")
                    nc.vector.tensor_scalar_add(osb[:], po[:], pb_t[:, 6 + mt : 7 + mt])
                    nc.sync.dma_start(outT[mt * P : (mt + 1) * P, oqsl], osb[:])

            def chain(qc, hp, inject=None):
                """S^T + exp chain for one (q-chunk, head-pair) unit.

                This is ScalarE's feeder; emitted as one contiguous
                high-priority block so nothing outranks the next S^T pair on
                the PE. Returns unit state for attv()/norm().

                inject: {ktp: [closure]} — emissions forced before that ktp's
                S^T (hard RAW ordering, e.g. the KT chunk it reads).
                """
                psA = psp.tile([P, QC], F32, tag="ab", bufs=4, name=f"att_a{hp}_{qc}")
                psB = psp.tile([P, QC], F32, tag="ab", bufs=4, name=f"att_b{hp}_{qc}")
                es = []
                qsl = slice(qc * QC, (qc + 1) * QC)
                for ktp in range(NKT // 2):
                    for f in (inject or {}).get(ktp, ()):
                        f()
                    stA = psp.tile([P, 2 * QC], F32, tag="st", bufs=2,
                                   name=f"stA{hp}_{qc}_{ktp}")
                    stB = psp.tile([P, 2 * QC], F32, tag="st", bufs=2,
                                   name=f"stB{hp}_{qc}_{ktp}")
                    for j in range(2):
                        kt = 2 * ktp + j
                        ksl = slice(kt * P, (kt + 1) * P)
                        jsl = slice(j * QC, (j + 1) * QC)
                        # S^T = K_h @ Q_h^T, two heads row-tiled in the PE
                        nc.tensor.matmul(
                            stA[:, jsl], KT[hp][0:DK, ksl], QT[hp][0:DK, qsl]
                        )
                        nc.tensor.matmul(
                            stB[:, jsl], KT[hp][DK:P, ksl], QT[hp][DK:P, qsl]
                        )
                    eA = wp.tile([P, 2 * QC], BF16, tag="E", bufs=32,
                                 name=f"eA{hp}_{qc}_{ktp}")
                    eB = wp.tile([P, 2 * QC], BF16, tag="E", bufs=32,
                                 name=f"eB{hp}_{qc}_{ktp}")
                    nc.scalar.activation(eA[:], stA[:], EXP, scale=1.0 / DK)
                    nc.scalar.activation(eB[:], stB[:], EXP, scale=1.0 / DK)
                    es.append((eA, eB))
                return {"qc": qc, "hp": hp, "psA": psA, "psB": psB, "es": es}

            def attv(u, lo, hi, v_pop=False):
                """attV accumulation for kv tiles [lo, hi) of unit u. Emitted
                after the NEXT unit's chain so the chain wins PE priority."""
                hA, hB = 2 * u["hp"], 2 * u["hp"] + 1
                for ktp in range(lo // 2, hi // 2):
                    eA, eB = u["es"][ktp]
                    if v_pop:
                        vproj(2 * ktp)
                        vproj(2 * ktp + 1)
                    for j in range(2):
                        kt = 2 * ktp + j
                        jsl = slice(j * QC, (j + 1) * QC)
                        nc.tensor.matmul(
                            u["psA"][:],
                            V[kt][:, hA * 2 * DK : (hA + 1) * 2 * DK],
                            eA[:, jsl],
                            start=(kt == 0),
                            stop=(kt == NKT - 1),
                            skip_group_check=True,
                        )
                        nc.tensor.matmul(
                            u["psB"][:],
                            V[kt][:, hB * 2 * DK : (hB + 1) * 2 * DK],
                            eB[:, jsl],
                            start=(kt == 0),
                            stop=(kt == NKT - 1),
                            skip_group_check=True,
                        )

            def norm(u):
                # attn rows = att * recip(denom); denom replicated in rows
                # 64-127 (ones cols of V). DVE only — no PE contention.
                qc, hp = u["qc"], u["hp"]
                qsl = slice(qc * QC, (qc + 1) * QC)
                nA = wp.tile([DK, QC], F32, tag="nm", bufs=4, name=f"nA{hp}_{qc}")
                nB = wp.tile([DK, QC], F32, tag="nm", bufs=4, name=f"nB{hp}_{qc}")
                nc.vector.reciprocal(nA[:], u["psA"][DK:P, :])
                nc.vector.reciprocal(nB[:], u["psB"][DK:P, :])
                nc.vector.tensor_mul(attn[hp][0:DK, qsl], u["psA"][0:DK, :], nA[:])
                nc.vector.tensor_mul(attn[hp][DK:P, qsl], u["psB"][0:DK, :], nB[:])

            # ---- emission schedule ----
            # hp-outer / qc-inner unit order. Chains are emitted ONE AHEAD of
            # their window's filler content: everything in window(w) is
            # emitted after chain(w+1), so the S^T -> exp stream always holds
            # top PE priority and filler work (attV halves, projections,
            # o-proj) soaks the PE idle underneath.
            units = [(qc, hp) for hp in range(NPT) for qc in range(NQC)]
            NU = len(units)
            us = []

            def window(w):
                # q-projection feeding chain(w+2), two windows ahead of use
                if w + 2 < NU:
                    qproj(units[w + 2][1], units[w + 2][0])
                # attV split in halves across two windows keeps each window's
                # PE load under the ScalarE window; from w==8 the units go
                # whole-attV so their norm + o-proj pull out of the tail
                if w > 0 and w <= 8:
                    attv(us[w - 1], 8, NKT, v_pop=(w == 1))
                    norm(us[w - 1])
                if w < 8:
                    attv(us[w], 0, 8, v_pop=(w == 0))
                else:
                    attv(us[w], 0, NKT)
                    norm(us[w])
                # fillers: k-projections a few units ahead of their chain,
                # o-proj once a q-chunk's attn is complete (hp==2 stream)
                if w == 2:
                    # all of K1 here: window(3) is emitted after chain(4),
                    # which already reads the full KT[1]
                    kproj(1, 0), kproj(1, 1), kproj(1, 2), kproj(1, 3)
                elif w == 5:
                    kproj(2, 0), kproj(2, 1)
                elif w == 6:
                    kproj(2, 2), kproj(2, 3)
                elif w == 8:
                    oproj(0)
                elif w == 9:
                    oproj(1)
                elif w == 10:
                    oproj(2)
                elif w == 11:
                    oproj(3)

            kproj(0, 0)
            qproj(0, 0)
            for i, (qc, hp) in enumerate(units):
                inject = None
                if i == 0:
                    inject = {2: [lambda: kproj(0, 1)],
                              4: [lambda: kproj(0, 2)],
                              6: [lambda: kproj(0, 3)]}
                us.append(chain(qc, hp, inject))
                if i == 0:
                    qproj(units[1][1], units[1][0])
                else:
                    window(i - 1)
            window(NU - 1)

    nc.compile()
    return nc


def make_in_maps(x, Wq, bq, Wk, bk, Wv, bv, Wo, bo):
    in_maps = []
    for c in range(NCORES):
        b, g = c // 2, c % 2
        sl = slice(g * GD, (g + 1) * GD)
        pbv = np.zeros((P, 12), np.float32)
        for j in range(NPT):
            pbv[:, 0 + j] = bq[sl][j * P : (j + 1) * P]
            pbv[:, 3 + j] = bk[sl][j * P : (j + 1) * P]
        if g == 0:
            for j in range(NXT):
                pbv[:, 6 + j] = bo[j * P : (j + 1) * P]
        rcbv = np.zeros((1, 512), NPBF16)
        rcbv[0, :GD] = bv[sl].astype(NPBF16)
        rcbv[0, GD : GD + P] = NPBF16(1.0)
        in_maps.append(
            {
                "xT": np.ascontiguousarray(x[b].T).astype(NPBF16),
                "wqT": np.ascontiguousarray(Wq[sl, :].T).astype(NPBF16),
                "wkT": np.ascontiguousarray(Wk[sl, :].T).astype(NPBF16),
                "wvT": np.ascontiguousarray(Wv[sl, :].T).astype(NPBF16),
                "woT": np.ascontiguousarray(Wo[:, sl].T).astype(NPBF16),
                "pb": pbv,
                "rcb": rcbv,
            }
        )
    return in_maps


def kernel(x, Wq, bq, Wk, bk, Wv, bv, Wo, bo, _trace=False):
    x = np.asarray(x, np.float32)
    args = [np.asarray(a, np.float32) for a in (Wq, bq, Wk, bk, Wv, bv, Wo, bo)]
    if "nc" not in _NC_CACHE:
        _NC_CACHE["nc"] = build_nc()
    nc = _NC_CACHE["nc"]
    in_maps = make_in_maps(x, *args)
    res = run_bass_kernel_spmd(
        nc, in_maps, core_ids=list(range(NCORES)), trace=_trace
    )
    _NC_CACHE["last_result"] = res
    out = np.empty((B, S, DM), np.float32)
    for b in range(B):
        out[b] = (
            np.asarray(res.results[2 * b]["outT"], dtype=np.float32)
            + np.asarray(res.results[2 * b + 1]["outT"], dtype=np.float32)
        ).T
    return out
